# revision 8
# baseline (speedup 1.0000x reference)
"""Trainium2 Bass kernel for nn_AttentionContextEncoder (gnn_message_passing).

reference:
  ents = ctx.T.reshape(B, 7, 4)
  prop_emb = relu(ents @ w_prop + b_prop)                      # [B,7,128]
  diffs[b,i,j,:] = ents[b,i,:] - ents[b,j,:]
  dist = sqrt(diffs[...,0]^2 + diffs[...,1]^2)
  rel = relu(concat([diffs, dist]) @ w_rel + b_rel)            # [B,7,7,128]
  rel_emb = sum_{j != i} rel[:, i, j, :]                       # [B,7,128]
  out = concat([prop_emb, rel_emb], -1)                        # [B,7,256]

Strategy: pure data parallelism over 8 cores (B=2048 each). Per core:
- 21 unordered pairs (i<j) get a 5-row rhs block [diffs(4); dist] in SBUF,
  at 32-aligned partition "strips" (4 strips x up to 8 free-dim pages).
- Pre-activations g_ij / g_ji come from K=5 matmuls against W+ = w_rel and
  W- = [-w0..-w3, w4] (strip-packed, weights replicated per strip).
- b_rel is folded in two ways: DVE route uses the max-shift identity
  relu(x+b) = max(x,-b)+b with a per-partition scalar and a 6*b seed;
  ACT route uses activation(Relu, bias=b_rel).
- Sum over j: DVE route accumulates with fused scalar_tensor_tensor
  (acc = max(g,-b) + acc); ACT route relu's to bf16 tiles that are summed
  in PSUM by identity matmuls.
- Output is written in device layout [2, 7, 128h, 2048b]; the host
  transposes to [B, 7, 256].
"""
import numpy as np
from contextlib import ExitStack

import concourse.bass as bass
import concourse.bacc as bacc
import concourse.mybir as mybir
import concourse.tile as tile
from concourse.bass_utils import run_bass_kernel_spmd

F32 = mybir.dt.float32
BF16 = mybir.dt.bfloat16
AF = mybir.ActivationFunctionType
ALU = mybir.AluOpType

NUM_ENT = 7
DIM_ENT = 4
H = 128
B_TOTAL = 16384
N_CORES = 8
B = B_TOTAL // N_CORES  # 2048 per core
HC = 1024               # half-chunk processed per epilogue op
USE_SWDGE = False       # SWDGE cast/accum DMAs (suspected HW-hang); off = DVE fallbacks

# ---- pair table: 21 unordered pairs (i<j) -> (strip, page) ----
# strip 0: i=0 pages 0-5 (j=1..6), i=4 pages 6-7 (j=5,6)
# strip 1: i=1 pages 0-4 (j=2..6), i=5 page 5 (j=6)
# strip 2: i=2 pages 0-3 (j=3..6)
# strip 3: i=3 pages 0-2 (j=4..6)
PAIRS = []  # (i, j, strip, page), index = strip-major order
for s, i0 in enumerate(range(4)):
    for g, j in enumerate(range(i0 + 1, 7)):
        PAIRS.append((i0, j, s, g))
PAIRS.sort(key=lambda t: (t[2], t[3]))
_extra = [(4, 5, 0, 6), (4, 6, 0, 7), (5, 6, 1, 5)]
PAIRS = [p for p in PAIRS] + _extra
PAIRS.sort(key=lambda t: (t[2], t[3]))
PAIR_IDX = {(i, j): k for k, (i, j, s, g) in enumerate(PAIRS)}
STRIP_PAGES = [0, 0, 0, 0]
for (_, _, s, g) in PAIRS:
    STRIP_PAGES[s] = max(STRIP_PAGES[s], g + 1)
NPAGES = max(STRIP_PAGES)

# prop slot for entity i: (strip, page)
PROP_SLOT = {i: (i, 0) for i in range(4)}
PROP_SLOT.update({i: (i - 4, 1) for i in range(4, 7)})

# epilogue routing
ACT_SET = {0, 2, 4, 6}   # rel_emb targets summed via ACT relu + identity matmuls
DVE_SET = {1, 3, 5}      # rel_emb targets via fused scalar_tensor_tensor
PROP_ACT = {1, 3, 5}     # prop relu on ACT
PROP_DVE = {0, 2, 4, 6}  # prop relu on DVE (max-shift)


def build():
    nc = bacc.Bacc("TRN2", target_bir_lowering=False, debug=False,
                   num_devices=N_CORES)
    ctx_d = nc.dram_tensor("ctx", [NUM_ENT * DIM_ENT, B], F32, kind="ExternalInput").ap()
    wrel_d = nc.dram_tensor("w_rel", [DIM_ENT + 1, H], F32, kind="ExternalInput").ap()
    brel_d = nc.dram_tensor("b_rel", [H, 1], F32, kind="ExternalInput").ap()
    wprop_d = nc.dram_tensor("w_prop", [DIM_ENT, H], F32, kind="ExternalInput").ap()
    bprop_d = nc.dram_tensor("b_prop", [H, 1], F32, kind="ExternalInput").ap()
    ident_d = nc.dram_tensor("ident", [H, H], F32, kind="ExternalInput").ap()
    out_d = nc.dram_tensor("out", [2, NUM_ENT, H, B], F32, kind="ExternalOutput").ap()

    with tile.TileContext(nc) as tc, ExitStack() as ctx:
        stat = ctx.enter_context(tc.tile_pool(name="stat", bufs=1))
        accp = ctx.enter_context(tc.tile_pool(name="accp", bufs=3))
        proppp = ctx.enter_context(tc.tile_pool(name="proppp", bufs=2))
        rpool = ctx.enter_context(tc.tile_pool(name="rpool", bufs=4))
        psg = ctx.enter_context(tc.tile_pool(name="psg", bufs=2, space="PSUM"))
        pss = ctx.enter_context(tc.tile_pool(name="pss", bufs=2, space="PSUM"))

        # ---------- load + prep ----------
        ctxb = stat.tile([NUM_ENT * DIM_ENT, B], BF16)
        if USE_SWDGE:
            nc.gpsimd.dma_start(ctxb[:], ctx_d[:])        # cast f32->bf16 in DMA
        else:
            ctxf = stat.tile([NUM_ENT * DIM_ENT, B], F32)
            nc.sync.dma_start(ctxf[:], ctx_d[:])
            nc.vector.tensor_copy(ctxb[:], ctxf[:])
        negctx = stat.tile([NUM_ENT * DIM_ENT, B], BF16)
        nc.vector.tensor_scalar_mul(negctx[:], ctxb[:], -1.0)

        ident = stat.tile([H, H], BF16)
        if USE_SWDGE:
            nc.gpsimd.dma_start(ident[:], ident_d[:])
        else:
            identf = stat.tile([H, H], F32)
            nc.sync.dma_start(identf[:], ident_d[:])
            nc.vector.tensor_copy(ident[:], identf[:])

        wrelf = stat.tile([DIM_ENT + 1, H], F32)
        nc.sync.dma_start(wrelf[:], wrel_d[:])
        wrelb = stat.tile([DIM_ENT + 1, H], BF16)
        nc.vector.tensor_copy(wrelb[:], wrelf[:])
        wminb = stat.tile([DIM_ENT + 1, H], BF16)
        nc.vector.tensor_copy(wminb[:], wrelb[:])
        nc.vector.tensor_scalar_mul(wminb[0:DIM_ENT, :], wminb[0:DIM_ENT, :], -1.0)

        wpropf = stat.tile([DIM_ENT, H], F32)
        nc.sync.dma_start(wpropf[:], wprop_d[:])
        wpropb = stat.tile([DIM_ENT, H], BF16)
        nc.vector.tensor_copy(wpropb[:], wpropf[:])

        # strip replicas of the weights
        wp_all = stat.tile([H, H], BF16)   # row 32s+k = W+ row k, per strip s
        wm_all = stat.tile([H, H], BF16)
        wq_all = stat.tile([H, H], BF16)
        for s in range(4):
            nc.sync.dma_start(wp_all[32 * s:32 * s + 5, :], wrelb[:, :])
            nc.sync.dma_start(wm_all[32 * s:32 * s + 5, :], wminb[:, :])
            nc.sync.dma_start(wq_all[32 * s:32 * s + 4, :], wpropb[:, :])

        brel = stat.tile([H, 1], F32)
        nc.sync.dma_start(brel[:], brel_d[:])
        nbrel = stat.tile([H, 1], F32)
        nc.scalar.mul(nbrel[:], brel[:], -1.0)
        brel6 = stat.tile([H, 1], F32)
        nc.scalar.mul(brel6[:], brel[:], 6.0)
        bprop = stat.tile([H, 1], F32)
        nc.sync.dma_start(bprop[:], bprop_d[:])
        nbprop = stat.tile([H, 1], F32)
        nc.scalar.mul(nbprop[:], bprop[:], -1.0)

        # rhs blocks: diffs
        rhs6 = stat.tile([H, NPAGES, B], BF16)
        prop6 = stat.tile([H, 2, B], BF16)
        if USE_SWDGE:
            # gather ctx_i, then CCE-accumulate -ctx_j in a second DMA
            for (i, j, s, g) in PAIRS:
                nc.sync.dma_start(rhs6[32 * s:32 * s + 4, g, :], ctxb[4 * i:4 * i + 4, :])
                nc.gpsimd.dma_start(rhs6[32 * s:32 * s + 4, g, :], negctx[4 * j:4 * j + 4, :],
                                    accum_op=ALU.add)
        else:
            # gather ctx_i / ctx_j into staging sets, one big DVE subtract
            ctxi6 = stat.tile([H, NPAGES, B], BF16)
            ctxj6 = stat.tile([H, NPAGES, B], BF16)
            nc.gpsimd.memset(ctxi6[:], 0.0)
            nc.gpsimd.memset(ctxj6[:], 0.0)
            for (i, j, s, g) in PAIRS:
                nc.sync.dma_start(ctxi6[32 * s:32 * s + 4, g, :], ctxb[4 * i:4 * i + 4, :])
                nc.sync.dma_start(ctxj6[32 * s:32 * s + 4, g, :], ctxb[4 * j:4 * j + 4, :])
            nc.vector.tensor_sub(rhs6[0:100, :, :], ctxi6[0:100, :, :], ctxj6[0:100, :, :])
        for i in range(NUM_ENT):
            ps_, pg_ = PROP_SLOT[i]
            nc.sync.dma_start(prop6[32 * ps_:32 * ps_ + 4, pg_, :], ctxb[4 * i:4 * i + 4, :])

        # dist: gather x/y diffs per pair -> [21, 2, B], square, add, sqrt, scatter
        ddp = stat.tile([21, 2, B], BF16)
        for k, (i, j, s, g) in enumerate(PAIRS):
            nc.sync.dma_start(ddp[k:k + 1, :, :], rhs6[32 * s:32 * s + 2, g, :])
        sq = stat.tile([21, 2, B], F32)
        nc.vector.tensor_mul(sq[:], ddp[:], ddp[:])
        d2 = stat.tile([21, B], F32)
        nc.vector.tensor_add(d2[:], sq[:, 0, :], sq[:, 1, :])
        dist = stat.tile([21, B], BF16)
        nc.scalar.activation(dist[:], d2[:], AF.Sqrt)
        base = 0
        for s in range(4):
            np_ = STRIP_PAGES[s]
            nc.sync.dma_start(rhs6[32 * s + 4:32 * s + 5, 0:np_, :], dist[base:base + np_, :])
            base += np_

        # ---------- main: per rel target t, sum_j relu(g_tj) ----------
        for t in range(NUM_ENT):
            acc = accp.tile([H, B], F32, tag="acc")
            others = [j for j in range(NUM_ENT) if j != t]
            for hcix in range(B // HC):
                c0 = hcix * HC
                if t in ACT_SET:
                    s0 = pss.tile([H, 512], F32, tag="s0")
                    s1 = pss.tile([H, 512], F32, tag="s1")
                for k, j in enumerate(others):
                    a, b_ = (t, j) if t < j else (j, t)
                    (_, _, s, g) = PAIRS[PAIR_IDX[(a, b_)]]
                    w_all = wp_all if t < j else wm_all
                    gt = psg.tile([H, HC], F32, tag="g")
                    for sub in range(HC // 512):
                        nc.tensor.matmul(
                            gt[:, 512 * sub:512 * sub + 512],
                            w_all[32 * s:32 * s + 5, :],
                            rhs6[32 * s:32 * s + 5, g, c0 + 512 * sub:c0 + 512 * sub + 512],
                            start=True, stop=True, tile_position=(32 * s, 0))
                    if t in DVE_SET:
                        in1 = brel6[:].broadcast_to([H, HC]) if k == 0 else acc[:, c0:c0 + HC]
                        nc.vector.scalar_tensor_tensor(
                            acc[:, c0:c0 + HC], gt[:], nbrel[:], in1,
                            op0=ALU.max, op1=ALU.add)
                    else:
                        r = rpool.tile([H, HC], BF16, tag="r")
                        nc.scalar.activation(r[:], gt[:], AF.Relu, bias=brel[:])
                        nc.tensor.matmul(s0[:], ident[:], r[:, 0:512],
                                         start=(k == 0), stop=(k == 5))
                        nc.tensor.matmul(s1[:], ident[:], r[:, 512:1024],
                                         start=(k == 0), stop=(k == 5))
                if t in ACT_SET:
                    nc.scalar.copy(acc[:, c0:c0 + 512], s0[:])
                    nc.scalar.copy(acc[:, c0 + 512:c0 + 1024], s1[:])
            nc.sync.dma_start(out_d[1, t, :, :], acc[:])

            # prop for the same t
            pacc = proppp.tile([H, B], F32, tag="pacc")
            ps_, pg_ = PROP_SLOT[t]
            for hcix in range(B // HC):
                c0 = hcix * HC
                pt = psg.tile([H, HC], F32, tag="g")
                for sub in range(HC // 512):
                    nc.tensor.matmul(
                        pt[:, 512 * sub:512 * sub + 512],
                        wq_all[32 * ps_:32 * ps_ + 4, :],
                        prop6[32 * ps_:32 * ps_ + 4, pg_, c0 + 512 * sub:c0 + 512 * sub + 512],
                        start=True, stop=True, tile_position=(32 * ps_, 0))
                if t in PROP_ACT:
                    nc.scalar.activation(pacc[:, c0:c0 + HC], pt[:], AF.Relu, bias=bprop[:])
                else:
                    nc.vector.scalar_tensor_tensor(
                        pacc[:, c0:c0 + HC], pt[:], nbprop[:],
                        bprop[:].broadcast_to([H, HC]),
                        op0=ALU.max, op1=ALU.add)
            nc.sync.dma_start(out_d[0, t, :, :], pacc[:])

    nc.compile()
    return nc


_NC_CACHE = None


def _get_nc():
    global _NC_CACHE
    if _NC_CACHE is None:
        _NC_CACHE = build()
    return _NC_CACHE


def run(ctx, w_prop, b_prop, w_rel, b_rel, trace=False):
    ctx = np.asarray(ctx, dtype=np.float32)
    nc = _get_nc()
    ident = np.eye(H, dtype=np.float32)
    shared = {
        "w_rel": np.asarray(w_rel, np.float32),
        "b_rel": np.asarray(b_rel, np.float32).reshape(H, 1),
        "w_prop": np.asarray(w_prop, np.float32),
        "b_prop": np.asarray(b_prop, np.float32).reshape(H, 1),
        "ident": ident,
    }
    in_maps = []
    for c in range(N_CORES):
        m = dict(shared)
        m["ctx"] = np.ascontiguousarray(ctx[:, c * B:(c + 1) * B])
        in_maps.append(m)
    res = run_bass_kernel_spmd(nc, in_maps, core_ids=list(range(N_CORES)),
                               trace=trace)
    shards = [np.asarray(res.results[c]["out"]) for c in range(N_CORES)]
    full = np.concatenate(shards, axis=3)                     # [2,7,128,16384]
    out = np.transpose(full, (3, 1, 0, 2)).reshape(B_TOTAL, NUM_ENT, 2 * H)
    return np.ascontiguousarray(out), res


def kernel(ctx, w_prop, b_prop, w_rel, b_rel):
    return run(ctx, w_prop, b_prop, w_rel, b_rel)[0]


# revision 10
# speedup vs baseline: 1.2616x; 1.2616x over previous
"""Trainium2 Bass kernel for nn_AttentionContextEncoder (gnn_message_passing).

reference:
  ents = ctx.T.reshape(B, 7, 4)
  prop_emb = relu(ents @ w_prop + b_prop)                      # [B,7,128]
  diffs[b,i,j,:] = ents[b,i,:] - ents[b,j,:]
  dist = sqrt(diffs[...,0]^2 + diffs[...,1]^2)
  rel = relu(concat([diffs, dist]) @ w_rel + b_rel)            # [B,7,7,128]
  rel_emb = sum_{j != i} rel[:, i, j, :]                       # [B,7,128]
  out = concat([prop_emb, rel_emb], -1)                        # [B,7,256]

Pure data parallelism over 8 cores (B=2048 each). Per core:
- 21 unordered pairs (i<j) hold a 5-row rhs block [diffs(4); dist] at
  32-aligned partition strips (4 strips x up to 6 free-dim pages).
- g_ij / g_ji come from K=5 matmuls vs W+ = w_rel and W- = [-w0..-w3, w4]
  (strip-packed via tile_position, weights replicated per strip).
- b_rel folding: DVE route uses relu(x+b) = max(x,-b)+b (per-partition
  scalar + 6b seed in the fused scalar_tensor_tensor chain); ACT route
  uses activation(Relu, bias=b_rel).
- Sum over j: DVE targets accumulate with fused STT straight from PSUM;
  ACT targets relu to bf16 SBUF tiles, summed by TT adds on DVE/GpSimd.
- Output in device layout [2, 7, 128h, 2048b]; host transposes.
"""
import numpy as np
from contextlib import ExitStack

import concourse.bass as bass
import concourse.bacc as bacc
import concourse.mybir as mybir
import concourse.tile as tile
from concourse.bass_utils import run_bass_kernel_spmd

F32 = mybir.dt.float32
BF16 = mybir.dt.bfloat16
AF = mybir.ActivationFunctionType
ALU = mybir.AluOpType

NUM_ENT = 7
DIM_ENT = 4
H = 128
B_TOTAL = 16384
N_CORES = 8
B = B_TOTAL // N_CORES  # 2048 per core

# ---- pair table: 21 unordered pairs (i<j) -> (strip, page), {6,5,5,5} ----
_ALLP = [(i, j) for i in range(NUM_ENT) for j in range(i + 1, NUM_ENT)]
PAIRS = []
_counts = [6, 5, 5, 5]
_k = 0
for s in range(4):
    for g in range(_counts[s]):
        i, j = _ALLP[_k]
        PAIRS.append((i, j, s, g))
        _k += 1
PAIR_IDX = {(i, j): k for k, (i, j, s, g) in enumerate(PAIRS)}
STRIP_PAGES = list(_counts)
NPAGES = max(STRIP_PAGES)

PROP_SLOT = {i: (i, 0) for i in range(4)}
PROP_SLOT.update({i: (i - 4, 1) for i in range(4, 7)})

# epilogue routing
DVE_SET = {5, 6}              # rel targets via fused STT from PSUM
ACT_SET = {0, 1, 2, 3, 4}     # rel targets via ACT relu -> bf16 r tiles
GPS_ADD = {0, 1}              # ACT targets whose TT adds run on GpSimd
PROP_ACT = {4}                # prop relu on ACT; the rest on DVE


def build(sim_init=False):
    nc = bacc.Bacc("TRN2", target_bir_lowering=False, debug=False,
                   num_devices=N_CORES)
    ctx_d = nc.dram_tensor("ctx", [NUM_ENT * DIM_ENT, B], F32, kind="ExternalInput").ap()
    wrel_d = nc.dram_tensor("w_rel", [DIM_ENT + 1, H], F32, kind="ExternalInput").ap()
    brel_d = nc.dram_tensor("b_rel", [H, 1], F32, kind="ExternalInput").ap()
    wprop_d = nc.dram_tensor("w_prop", [DIM_ENT, H], F32, kind="ExternalInput").ap()
    bprop_d = nc.dram_tensor("b_prop", [H, 1], F32, kind="ExternalInput").ap()
    out_d = nc.dram_tensor("out", [2, NUM_ENT, H, B], F32, kind="ExternalOutput").ap()

    with tile.TileContext(nc) as tc, ExitStack() as ctx:
        stat = ctx.enter_context(tc.tile_pool(name="stat", bufs=1))
        accp = ctx.enter_context(tc.tile_pool(name="accp", bufs=2))
        rpool = ctx.enter_context(tc.tile_pool(name="rpool", bufs=7))
        psg = ctx.enter_context(tc.tile_pool(name="psg", bufs=2, space="PSUM"))

        # ---------- load + prep ----------
        ctxf = stat.tile([NUM_ENT * DIM_ENT, B], F32)
        nc.sync.dma_start(ctxf[:], ctx_d[:])
        ctxb = stat.tile([NUM_ENT * DIM_ENT, B], BF16)
        nc.vector.tensor_copy(ctxb[:], ctxf[:])
        negctx = stat.tile([NUM_ENT * DIM_ENT, B], BF16)
        nc.vector.tensor_scalar_mul(negctx[:], ctxb[:], -1.0)

        wrelf = stat.tile([DIM_ENT + 1, H], F32)
        nc.sync.dma_start(wrelf[:], wrel_d[:])
        wrelb = stat.tile([DIM_ENT + 1, H], BF16)
        nc.vector.tensor_copy(wrelb[:], wrelf[:])
        wminb = stat.tile([DIM_ENT + 1, H], BF16)
        nc.vector.tensor_copy(wminb[:], wrelb[:])
        nc.vector.tensor_scalar_mul(wminb[0:DIM_ENT, :], wminb[0:DIM_ENT, :], -1.0)
        wpropf = stat.tile([DIM_ENT, H], F32)
        nc.sync.dma_start(wpropf[:], wprop_d[:])
        wpropb = stat.tile([DIM_ENT, H], BF16)
        nc.vector.tensor_copy(wpropb[:], wpropf[:])

        wp_all = stat.tile([H, H], BF16)
        wm_all = stat.tile([H, H], BF16)
        wq_all = stat.tile([H, H], BF16)
        for s in range(4):
            nc.sync.dma_start(wp_all[32 * s:32 * s + 5, :], wrelb[:, :])
            nc.sync.dma_start(wm_all[32 * s:32 * s + 5, :], wminb[:, :])
            nc.sync.dma_start(wq_all[32 * s:32 * s + 4, :], wpropb[:, :])

        brel = stat.tile([H, 1], F32)
        nc.sync.dma_start(brel[:], brel_d[:])
        nbrel = stat.tile([H, 1], F32)
        nc.scalar.mul(nbrel[:], brel[:], -1.0)
        brel6 = stat.tile([H, 1], F32)
        nc.scalar.mul(brel6[:], brel[:], 6.0)
        bprop = stat.tile([H, 1], F32)
        nc.sync.dma_start(bprop[:], bprop_d[:])
        nbprop = stat.tile([H, 1], F32)
        nc.scalar.mul(nbprop[:], bprop[:], -1.0)

        # staging gathers: spread issue over sync / scalar / gpsimd sequencers
        rhs6 = stat.tile([H, NPAGES, B], BF16)   # ctx_i staging, then diffs in-place
        prop6 = stat.tile([H, 2, B], BF16)
        ctxi6 = rhs6
        ctxj6 = stat.tile([H, NPAGES, B], BF16)
        if sim_init:
            nc.gpsimd.memset(ctxi6[:], 0.0)
            nc.gpsimd.memset(ctxj6[:], 0.0)
        _dmaeng = [nc.sync, nc.scalar, nc.gpsimd]
        for k, (i, j, s, g) in enumerate(PAIRS):
            e0 = _dmaeng[k % 3]
            e1 = _dmaeng[(k + 1) % 3]
            e0.dma_start(ctxi6[32 * s:32 * s + 4, g, :], ctxb[4 * i:4 * i + 4, :])
            e1.dma_start(ctxj6[32 * s:32 * s + 4, g, :], ctxb[4 * j:4 * j + 4, :])
        for i in range(NUM_ENT):
            ps_, pg_ = PROP_SLOT[i]
            _dmaeng[i % 3].dma_start(prop6[32 * ps_:32 * ps_ + 4, pg_, :],
                                     ctxb[4 * i:4 * i + 4, :])

        # diffs + dist, split by b-halves so the main loop can start early
        ddp = stat.tile([21, 2, B], BF16)
        dist = stat.tile([21, B], BF16)
        for h0 in (0, B // 2):
            hs = slice(h0, h0 + B // 2)
            nc.vector.tensor_sub(rhs6[0:100, :, hs], rhs6[0:100, :, hs],
                                 ctxj6[0:100, :, hs])
            for k, (i, j, s, g) in enumerate(PAIRS):
                _dmaeng[k % 3].dma_start(ddp[k:k + 1, :, hs],
                                         rhs6[32 * s:32 * s + 2, g, hs])
            sq = stat.tile([21, 2, B // 2], F32, tag="sq")
            nc.vector.tensor_mul(sq[:], ddp[:, :, hs], ddp[:, :, hs])
            d2 = stat.tile([21, B // 2], F32, tag="d2")
            nc.vector.tensor_add(d2[:], sq[:, 0, :], sq[:, 1, :])
            nc.scalar.activation(dist[:, hs], d2[:], AF.Sqrt)
            base = 0
            for s in range(4):
                np_ = STRIP_PAGES[s]
                _dmaeng[s % 3].dma_start(rhs6[32 * s + 4:32 * s + 5, 0:np_, hs],
                                         dist[base:base + np_, hs])
                base += np_

        def g_matmul(t, j):
            a, b_ = (t, j) if t < j else (j, t)
            (_, _, s, g) = PAIRS[PAIR_IDX[(a, b_)]]
            w_all = wp_all if t < j else wm_all
            gt = psg.tile([H, B], F32, tag="g")
            for sub in range(B // 512):
                nc.tensor.matmul(
                    gt[:, 512 * sub:512 * sub + 512],
                    w_all[32 * s:32 * s + 5, :],
                    rhs6[32 * s:32 * s + 5, g, 512 * sub:512 * sub + 512],
                    start=True, stop=True, tile_position=(32 * s, 0))
            return gt

        # ---------- main: per target t ----------
        for t in range(NUM_ENT):
            acc = accp.tile([H, B], F32, tag="acc")
            others = [j for j in range(NUM_ENT) if j != t]
            if t in ACT_SET:
                rts = []
                for j in others:
                    gt = g_matmul(t, j)
                    r = rpool.tile([H, B], BF16, tag="r")
                    nc.scalar.activation(r[:], gt[:], AF.Relu, bias=brel[:])
                    rts.append(r)
                eng = nc.gpsimd if t in GPS_ADD else nc.vector
                tmp = accp.tile([H, B], BF16, tag="tmpacc")
                eng.tensor_add(tmp[:], rts[0][:], rts[1][:])
                for r in rts[2:-1]:
                    eng.tensor_add(tmp[:], tmp[:], r[:])
                eng.tensor_add(acc[:], tmp[:], rts[-1][:])
            else:
                for k, j in enumerate(others):
                    gt = g_matmul(t, j)
                    in1 = brel6[:].broadcast_to([H, B]) if k == 0 else acc[:]
                    nc.vector.scalar_tensor_tensor(
                        acc[:], gt[:], nbrel[:], in1,
                        op0=ALU.max, op1=ALU.add)
            nc.sync.dma_start(out_d[1, t, :, :], acc[:])

            # prop for the same t
            pacc = accp.tile([H, B], F32, tag="pacc")
            ps_, pg_ = PROP_SLOT[t]
            pt = psg.tile([H, B], F32, tag="g")
            for sub in range(B // 512):
                nc.tensor.matmul(
                    pt[:, 512 * sub:512 * sub + 512],
                    wq_all[32 * ps_:32 * ps_ + 4, :],
                    prop6[32 * ps_:32 * ps_ + 4, pg_, 512 * sub:512 * sub + 512],
                    start=True, stop=True, tile_position=(32 * ps_, 0))
            if t in PROP_ACT:
                nc.scalar.activation(pacc[:], pt[:], AF.Relu, bias=bprop[:])
            else:
                nc.vector.scalar_tensor_tensor(
                    pacc[:], pt[:], nbprop[:], bprop[:].broadcast_to([H, B]),
                    op0=ALU.max, op1=ALU.add)
            nc.sync.dma_start(out_d[0, t, :, :], pacc[:])

    nc.compile()
    return nc


_NC_CACHE = None


def _get_nc():
    global _NC_CACHE
    if _NC_CACHE is None:
        _NC_CACHE = build()
    return _NC_CACHE


def run(ctx, w_prop, b_prop, w_rel, b_rel, trace=False):
    ctx = np.asarray(ctx, dtype=np.float32)
    nc = _get_nc()
    shared = {
        "w_rel": np.asarray(w_rel, np.float32),
        "b_rel": np.asarray(b_rel, np.float32).reshape(H, 1),
        "w_prop": np.asarray(w_prop, np.float32),
        "b_prop": np.asarray(b_prop, np.float32).reshape(H, 1),
    }
    in_maps = []
    for c in range(N_CORES):
        m = dict(shared)
        m["ctx"] = np.ascontiguousarray(ctx[:, c * B:(c + 1) * B])
        in_maps.append(m)
    res = run_bass_kernel_spmd(nc, in_maps, core_ids=list(range(N_CORES)),
                               trace=trace)
    shards = [np.asarray(res.results[c]["out"]) for c in range(N_CORES)]
    full = np.concatenate(shards, axis=3)                     # [2,7,128,16384]
    out = np.transpose(full, (3, 1, 0, 2)).reshape(B_TOTAL, NUM_ENT, 2 * H)
    return np.ascontiguousarray(out), res


def kernel(ctx, w_prop, b_prop, w_rel, b_rel):
    return run(ctx, w_prop, b_prop, w_rel, b_rel)[0]
